# revision 24
# baseline (speedup 1.0000x reference)
"""Bahdanau attention Trainium2 kernel.

Problem sizes (hardcoded): B=32, T=4096, D_ENC=D_DEC=512, D_ATT=256.
Sharding: data-parallel over batch across 8 NeuronCores (4 batches/core);
params replicated.

Per-core dataflow (per batch b):
  enc (4096,512) f32 --SWDGE cast DMA--> nat bf16 [t%128, t//128, e]
  PE is_transpose matmuls -> PSUM -> DVE copies -> encT bf16 [e%128, e//128, t]
  PE matmuls (W_encT stationary, bf16) -> psum [a,t] -> ACT tanh(+u bias)
    -> energy bf16 [a%128, a//128, t]
  scores^T: PE matmuls (energy chunks stationary, v moving) -> psum [t%128, 32]
  softmax on [128,32]: ACT exp(+accum rowsums), PE partition-sum + broadcast,
    DVE reciprocal/scale (weights bounded: |score| <= ||v||_1, exp safe in f32)
  weights out: fp32 PE transpose -> [32,128] -> DMA
  context: PE matmuls (exp(scores) columns stationary, nat moving) -> psum
    [1,512] -> ACT copy*(1/sum) -> DMA
"""

import sys

if "/opt/trn_rl_repo" not in sys.path:
    sys.path.insert(0, "/opt/trn_rl_repo")

from contextlib import ExitStack

import numpy as np
import ml_dtypes

import concourse.bass as bass
import concourse.mybir as mybir
import concourse.tile as tile
from concourse import bacc
from concourse import bass_utils

BF16 = mybir.dt.bfloat16
F32 = mybir.dt.float32

B, T, D_ENC, D_DEC, D_ATT = 32, 4096, 512, 512, 256
N_CORES = 8
B_LOC = B // N_CORES  # 4

_program_cache: dict = {}


def build_program(b_loc: int = B_LOC, t_len: int = T, apply_mask: bool = False):
    key = (b_loc, t_len, apply_mask)
    if key in _program_cache:
        return _program_cache[key]

    NT = t_len // 128       # 128-row t-blocks (32)
    NTT = t_len // 512      # 512-wide t-tiles (8)
    NEC = D_ENC // 128      # e-chunks (4)
    NAC = D_ATT // 128      # a-chunks (2)
    NDC = D_DEC // 128      # d-chunks (4)

    nc = bacc.Bacc("TRN2", target_bir_lowering=False, debug=False,
                   num_devices=N_CORES)

    enc_d = nc.dram_tensor("enc", (b_loc, t_len, D_ENC), F32, kind="ExternalInput")
    idnbf_d = nc.dram_tensor("idnbf", (128, 128), BF16, kind="ExternalInput")
    idnf_d = nc.dram_tensor("idnf", (128, 128), F32, kind="ExternalInput")
    dect_d = nc.dram_tensor("dect", (D_DEC, b_loc), F32, kind="ExternalInput")
    wencT_d = nc.dram_tensor("wencT", (D_ENC, D_ATT), BF16, kind="ExternalInput")
    wdecT_d = nc.dram_tensor("wdecT", (D_DEC, D_ATT), F32, kind="ExternalInput")
    bsum_d = nc.dram_tensor("bsum", (128, NAC), F32, kind="ExternalInput")
    v_d = nc.dram_tensor("v", (128, NAC), F32, kind="ExternalInput")
    if apply_mask:
        mbias_d = nc.dram_tensor("mbias", (b_loc, 128, NT), F32, kind="ExternalInput")
    ctx_d = nc.dram_tensor("ctx", (b_loc, D_ENC), F32, kind="ExternalOutput")
    wout_d = nc.dram_tensor("wout", (b_loc, t_len), F32, kind="ExternalOutput")

    with tile.TileContext(nc) as tc, ExitStack() as es:
        singles = es.enter_context(tc.tile_pool(name="singles", bufs=1))
        nat_pool = es.enter_context(tc.tile_pool(name="nat", bufs=2))
        encT_pool = es.enter_context(tc.tile_pool(name="encT", bufs=2))
        en_pool = es.enter_context(tc.tile_pool(name="energy", bufs=2))
        scr_pool = es.enter_context(tc.tile_pool(name="scratch", bufs=1))
        small_pool = es.enter_context(tc.tile_pool(name="small", bufs=2))
        tr_psum = es.enter_context(tc.tile_pool(name="trps", bufs=2, space="PSUM"))
        ep_psum = es.enter_context(tc.tile_pool(name="epps", bufs=3, space="PSUM"))
        sc_psum = es.enter_context(tc.tile_pool(name="scps", bufs=1, space="PSUM"))
        cx_psum = es.enter_context(tc.tile_pool(name="cxps", bufs=1, space="PSUM"))
        sm_psum = es.enter_context(tc.tile_pool(name="smps", bufs=1, space="PSUM"))

        # ---- constants (identities via DMA: keeps the gpsimd queue free
        # for the cast-load DMAs) ----
        idn_bf = singles.tile([128, 128], BF16)
        nc.sync.dma_start(out=idn_bf, in_=idnbf_d.ap())
        idn_f32 = singles.tile([128, 128], F32)
        nc.sync.dma_start(out=idn_f32, in_=idnf_d.ap())
        ones_f32 = singles.tile([128, 1], F32)
        nc.vector.memset(ones_f32, 1.0)
        ones_bf = singles.tile([128, 1], BF16)
        nc.vector.memset(ones_bf, 1.0)
        ones_row = singles.tile([1, 128], F32)
        nc.vector.memset(ones_row, 1.0)

        wencT_sb = singles.tile([128, NEC, D_ATT], BF16)
        nc.sync.dma_start(out=wencT_sb,
                          in_=wencT_d.ap().rearrange("(c p) a -> p c a", p=128))
        wdecT_sb = singles.tile([128, NDC, D_ATT], F32)
        nc.sync.dma_start(out=wdecT_sb,
                          in_=wdecT_d.ap().rearrange("(c p) a -> p c a", p=128))
        bsum_sb = singles.tile([128, NAC], F32)
        nc.sync.dma_start(out=bsum_sb, in_=bsum_d.ap())
        v_sb = singles.tile([128, NAC], F32)
        nc.sync.dma_start(out=v_sb, in_=v_d.ap())
        v_bf = singles.tile([128, NAC], BF16)
        nc.vector.tensor_copy(v_bf, v_sb)
        dec_sb = singles.tile([128, NDC, b_loc], F32)
        nc.sync.dma_start(out=dec_sb,
                          in_=dect_d.ap().rearrange("(c p) b -> p c b", p=128))

        # ---- u[a, b] = W_dec @ h_b + b_dec + b_enc (tanh bias) ----
        u_sb = singles.tile([128, NAC, b_loc], F32)
        for ac in range(NAC):
            ups = sm_psum.tile([128, 128], F32, tag="smp")
            for dc in range(NDC):
                nc.tensor.matmul(ups[:, :b_loc],
                                 lhsT=wdecT_sb[:, dc, ac * 128:(ac + 1) * 128],
                                 rhs=dec_sb[:, dc, :],
                                 start=(dc == 0), stop=(dc == NDC - 1))
            nc.scalar.activation(out=u_sb[:, ac, :], in_=ups[:, :b_loc],
                                 func=mybir.ActivationFunctionType.Identity,
                                 bias=bsum_sb[:, ac:ac + 1], scale=1.0)

        for b in range(b_loc):
            # ---- load + cast (split so transposes start on partial data) ----
            nat = nat_pool.tile([128, NT, D_ENC], BF16, tag="nat")
            enc_b = enc_d.ap()[b].rearrange("(n p) e -> p n e", p=128)
            if b == 0:
                splits = [1, 1, 2, 4, 8, 8, 8]
            else:
                splits = [4] * 8
            pos = 0
            for w in splits:
                nc.gpsimd.dma_start(
                    out=nat[:, pos:pos + w, :],
                    in_=enc_b[:, pos:pos + w, :])
                pos += w

            # ---- transpose to encT ----
            encT = encT_pool.tile([128, NEC, t_len], BF16, tag="encT")
            for tt in range(NTT):
                for ec in range(NEC):
                    ps = tr_psum.tile([128, 512], BF16, tag="trps")
                    for q in range(4):
                        nc.tensor.transpose(
                            ps[:, q * 128:(q + 1) * 128],
                            nat[:, tt * 4 + q, ec * 128:(ec + 1) * 128],
                            idn_bf)
                    nc.vector.tensor_copy(
                        out=encT[:, ec, tt * 512:(tt + 1) * 512], in_=ps)

            # ---- enc_proj + tanh -> energy ----
            energy = en_pool.tile([128, NAC, t_len], BF16, tag="energy")
            for tt in range(NTT):
                for ac in range(NAC):
                    pps = ep_psum.tile([128, 512], F32, tag="epps")
                    for ec in range(NEC):
                        nc.tensor.matmul(
                            pps,
                            lhsT=wencT_sb[:, ec, ac * 128:(ac + 1) * 128],
                            rhs=encT[:, ec, tt * 512:(tt + 1) * 512],
                            start=(ec == 0), stop=(ec == NEC - 1))
                    nc.scalar.activation(
                        out=energy[:, ac, tt * 512:(tt + 1) * 512], in_=pps,
                        func=mybir.ActivationFunctionType.Tanh,
                        bias=u_sb[:, ac, b:b + 1], scale=1.0)

            # ---- scores^T [t%128, c] ----
            scps = sc_psum.tile([128, NT], F32, tag="scps")
            for c in range(NT):
                for ac in range(NAC):
                    nc.tensor.matmul(
                        scps[:, c:c + 1],
                        lhsT=energy[:, ac, c * 128:(c + 1) * 128],
                        rhs=v_bf[:, ac:ac + 1],
                        start=(ac == 0), stop=(ac == NAC - 1))

            # ---- softmax on [128, NT] ----
            p_sb = small_pool.tile([128, NT], F32, tag="p")
            rowsum = small_pool.tile([128, 1], F32, tag="rowsum")
            if apply_mask:
                mb = small_pool.tile([128, NT], F32, tag="mb")
                nc.sync.dma_start(out=mb, in_=mbias_d.ap()[b])
                nc.vector.tensor_add(p_sb, scps, mb)
                nc.scalar.activation(out=p_sb, in_=p_sb,
                                     func=mybir.ActivationFunctionType.Exp,
                                     accum_out=rowsum)
            else:
                nc.scalar.activation(out=p_sb, in_=scps,
                                     func=mybir.ActivationFunctionType.Exp,
                                     accum_out=rowsum)
            # total = sum over partitions of rowsum (PE), then 1/total
            tot_ps = sm_psum.tile([128, 128], F32, tag="smp")
            nc.tensor.matmul(tot_ps[:1, :1], lhsT=rowsum, rhs=ones_f32,
                             start=True, stop=True)
            sinv = small_pool.tile([1, 1], F32, tag="sinv")
            nc.vector.reciprocal(sinv, tot_ps[:1, :1])
            # broadcast 1/total to all partitions
            bc_ps = sm_psum.tile([128, 128], F32, tag="smp")
            nc.tensor.matmul(bc_ps[:, :1], lhsT=ones_row, rhs=sinv,
                             start=True, stop=True)
            sinv_bc = small_pool.tile([128, 1], F32, tag="sinvbc")
            nc.vector.tensor_copy(sinv_bc, bc_ps[:, :1])

            # ---- weights out: W = P * sinv, transpose, DMA ----
            w_sb = small_pool.tile([128, NT], F32, tag="w")
            nc.vector.tensor_scalar_mul(w_sb, p_sb, sinv_bc)
            wt_ps = sm_psum.tile([128, 128], F32, tag="smp")
            nc.tensor.transpose(wt_ps[:NT, :], w_sb, idn_f32)
            wT_sb = small_pool.tile([NT, 128], F32, tag="wT")
            nc.vector.tensor_copy(wT_sb, wt_ps[:NT, :])
            nc.sync.dma_start(
                out=wout_d.ap()[b].rearrange("(c p) -> c p", c=NT), in_=wT_sb)

            # ---- context: sum_t exp(s_t) * enc[t, :], scaled by sinv ----
            p_bf = small_pool.tile([128, NT], BF16, tag="pbf")
            nc.vector.tensor_copy(p_bf, p_sb)
            cxps = cx_psum.tile([1, D_ENC], F32, tag="cxps")
            for c in range(NT):
                nc.tensor.matmul(cxps, lhsT=p_bf[:, c:c + 1], rhs=nat[:, c, :],
                                 start=(c == 0), stop=(c == NT - 1))
            ctx_sb = small_pool.tile([1, D_ENC], F32, tag="ctx")
            nc.scalar.activation(out=ctx_sb, in_=cxps,
                                 func=mybir.ActivationFunctionType.Copy,
                                 scale=sinv, bias=0.0)
            nc.sync.dma_start(out=ctx_d.ap()[b:b + 1, :], in_=ctx_sb)

    nc.compile()
    _program_cache[key] = nc
    return nc


def _prep_shared(W_enc, b_enc, W_dec, b_dec, v):
    wencT = np.ascontiguousarray(W_enc.T).astype(ml_dtypes.bfloat16)
    wdecT = np.ascontiguousarray(W_dec.T).astype(np.float32)
    bsum = np.ascontiguousarray(
        (b_enc.astype(np.float32) + b_dec.astype(np.float32))
        .reshape(D_ATT // 128, 128).T)
    v2 = np.ascontiguousarray(
        v.astype(np.float32).reshape(D_ATT // 128, 128).T)
    return wencT, wdecT, bsum, v2


def kernel_impl(decoder_hidden, encoder_outputs, encoder_mask,
                W_enc, b_enc, W_dec, b_dec, v, trace=False, trace_kwargs=None):
    decoder_hidden = np.asarray(decoder_hidden, dtype=np.float32)
    encoder_outputs = np.asarray(encoder_outputs, dtype=np.float32)
    encoder_mask = np.asarray(encoder_mask)
    W_enc = np.asarray(W_enc, dtype=np.float32)
    b_enc = np.asarray(b_enc, dtype=np.float32)
    W_dec = np.asarray(W_dec, dtype=np.float32)
    b_dec = np.asarray(b_dec, dtype=np.float32)
    v = np.asarray(v, dtype=np.float32)

    apply_mask = not bool(encoder_mask.all())
    nc = build_program(B_LOC, T, apply_mask)
    wencT, wdecT, bsum, v2 = _prep_shared(W_enc, b_enc, W_dec, b_dec, v)

    in_maps = []
    for c in range(N_CORES):
        lo, hi = c * B_LOC, (c + 1) * B_LOC
        m = {
            "enc": np.ascontiguousarray(encoder_outputs[lo:hi]),
            "idnbf": np.eye(128, dtype=ml_dtypes.bfloat16),
            "idnf": np.eye(128, dtype=np.float32),
            "dect": np.ascontiguousarray(decoder_hidden[lo:hi].T),
            "wencT": wencT,
            "wdecT": wdecT,
            "bsum": bsum,
            "v": v2,
        }
        if apply_mask:
            mbias = np.where(encoder_mask[lo:hi], 0.0, -1e30).astype(np.float32)
            # scores^T layout: [b, t%128, t//128]
            m["mbias"] = np.ascontiguousarray(
                mbias.reshape(B_LOC, T // 128, 128).transpose(0, 2, 1))
        in_maps.append(m)

    res = bass_utils.run_bass_kernel_spmd(
        nc, in_maps, core_ids=list(range(N_CORES)), trace=trace,
        **(trace_kwargs or {}))
    ctx = np.concatenate([res.results[c]["ctx"] for c in range(N_CORES)], axis=0)
    wts = np.concatenate([res.results[c]["wout"] for c in range(N_CORES)], axis=0)
    return (ctx, wts), res


def kernel(decoder_hidden, encoder_outputs, encoder_mask,
           W_enc, b_enc, W_dec, b_dec, v):
    out, _ = kernel_impl(decoder_hidden, encoder_outputs, encoder_mask,
                         W_enc, b_enc, W_dec, b_dec, v)
    return out


# revision 25
# speedup vs baseline: 1.0558x; 1.0558x over previous
"""Bahdanau attention Trainium2 kernel.

Problem sizes (hardcoded): B=32, T=4096, D_ENC=D_DEC=512, D_ATT=256.
Sharding: data-parallel over batch across 8 NeuronCores (4 batches/core);
params replicated.

Per-core dataflow (per batch b):
  enc (4096,512) f32 --SWDGE cast DMA--> nat bf16 [t%128, t//128, e]
  PE is_transpose matmuls -> PSUM -> DVE copies -> encT bf16 [e%128, e//128, t]
  PE matmuls (W_encT stationary, bf16) -> psum [a,t] -> ACT tanh(+u bias)
    -> energy bf16 [a%128, a//128, t]
  scores^T: PE matmuls (energy chunks stationary, v moving) -> psum [t%128, 32]
  softmax on [128,32]: ACT exp(+accum rowsums), PE partition-sum + broadcast,
    DVE reciprocal/scale (weights bounded: |score| <= ||v||_1, exp safe in f32)
  weights out: fp32 PE transpose -> [32,128] -> DMA
  context: PE matmuls (exp(scores) columns stationary, nat moving) -> psum
    [1,512] -> ACT copy*(1/sum) -> DMA
"""

import sys

if "/opt/trn_rl_repo" not in sys.path:
    sys.path.insert(0, "/opt/trn_rl_repo")

from contextlib import ExitStack

import numpy as np
import ml_dtypes

import concourse.bass as bass
import concourse.mybir as mybir
import concourse.tile as tile
from concourse import bacc
from concourse import bass_utils

BF16 = mybir.dt.bfloat16
F32 = mybir.dt.float32

B, T, D_ENC, D_DEC, D_ATT = 32, 4096, 512, 512, 256
N_CORES = 8
B_LOC = B // N_CORES  # 4

_program_cache: dict = {}


def build_program(b_loc: int = B_LOC, t_len: int = T, apply_mask: bool = False):
    key = (b_loc, t_len, apply_mask)
    if key in _program_cache:
        return _program_cache[key]

    NT = t_len // 128       # 128-row t-blocks (32)
    NTT = t_len // 512      # 512-wide t-tiles (8)
    NEC = D_ENC // 128      # e-chunks (4)
    NAC = D_ATT // 128      # a-chunks (2)
    NDC = D_DEC // 128      # d-chunks (4)

    nc = bacc.Bacc("TRN2", target_bir_lowering=False, debug=False,
                   num_devices=N_CORES)

    enc_d = nc.dram_tensor("enc", (b_loc, t_len, D_ENC), F32, kind="ExternalInput")
    idnbf_d = nc.dram_tensor("idnbf", (128, 128), BF16, kind="ExternalInput")
    idnf_d = nc.dram_tensor("idnf", (128, 128), F32, kind="ExternalInput")
    dect_d = nc.dram_tensor("dect", (D_DEC, b_loc), F32, kind="ExternalInput")
    wencT_d = nc.dram_tensor("wencT", (D_ENC, D_ATT), BF16, kind="ExternalInput")
    wdecT_d = nc.dram_tensor("wdecT", (D_DEC, D_ATT), F32, kind="ExternalInput")
    bsum_d = nc.dram_tensor("bsum", (128, NAC), F32, kind="ExternalInput")
    v_d = nc.dram_tensor("v", (128, NAC), F32, kind="ExternalInput")
    if apply_mask:
        mbias_d = nc.dram_tensor("mbias", (b_loc, 128, NT), F32, kind="ExternalInput")
    ctx_d = nc.dram_tensor("ctx", (b_loc, D_ENC), F32, kind="ExternalOutput")
    wout_d = nc.dram_tensor("wout", (b_loc, t_len), F32, kind="ExternalOutput")

    with tile.TileContext(nc) as tc, ExitStack() as es:
        singles = es.enter_context(tc.tile_pool(name="singles", bufs=1))
        nat_pool = es.enter_context(tc.tile_pool(name="nat", bufs=2))
        encT_pool = es.enter_context(tc.tile_pool(name="encT", bufs=2))
        en_pool = es.enter_context(tc.tile_pool(name="energy", bufs=2))
        scr_pool = es.enter_context(tc.tile_pool(name="scratch", bufs=1))
        small_pool = es.enter_context(tc.tile_pool(name="small", bufs=2))
        tr_psum = es.enter_context(tc.tile_pool(name="trps", bufs=3, space="PSUM"))
        ep_psum = es.enter_context(tc.tile_pool(name="epps", bufs=2, space="PSUM"))
        sc_psum = es.enter_context(tc.tile_pool(name="scps", bufs=1, space="PSUM"))
        cx_psum = es.enter_context(tc.tile_pool(name="cxps", bufs=1, space="PSUM"))
        sm_psum = es.enter_context(tc.tile_pool(name="smps", bufs=1, space="PSUM"))

        # ---- constants (identities via DMA: keeps the gpsimd queue free
        # for the cast-load DMAs) ----
        idn_bf = singles.tile([128, 128], BF16)
        nc.sync.dma_start(out=idn_bf, in_=idnbf_d.ap())
        idn_f32 = singles.tile([128, 128], F32)
        nc.sync.dma_start(out=idn_f32, in_=idnf_d.ap())
        ones_f32 = singles.tile([128, 1], F32)
        nc.vector.memset(ones_f32, 1.0)
        ones_bf = singles.tile([128, 1], BF16)
        nc.vector.memset(ones_bf, 1.0)
        ones_row = singles.tile([1, 128], F32)
        nc.vector.memset(ones_row, 1.0)

        wencT_sb = singles.tile([128, NEC, D_ATT], BF16)
        nc.sync.dma_start(out=wencT_sb,
                          in_=wencT_d.ap().rearrange("(c p) a -> p c a", p=128))
        wdecT_sb = singles.tile([128, NDC, D_ATT], F32)
        nc.sync.dma_start(out=wdecT_sb,
                          in_=wdecT_d.ap().rearrange("(c p) a -> p c a", p=128))
        bsum_sb = singles.tile([128, NAC], F32)
        nc.sync.dma_start(out=bsum_sb, in_=bsum_d.ap())
        v_sb = singles.tile([128, NAC], F32)
        nc.sync.dma_start(out=v_sb, in_=v_d.ap())
        v_bf = singles.tile([128, NAC], BF16)
        nc.vector.tensor_copy(v_bf, v_sb)
        dec_sb = singles.tile([128, NDC, b_loc], F32)
        nc.sync.dma_start(out=dec_sb,
                          in_=dect_d.ap().rearrange("(c p) b -> p c b", p=128))

        # ---- u[a, b] = W_dec @ h_b + b_dec + b_enc (tanh bias) ----
        u_sb = singles.tile([128, NAC, b_loc], F32)
        for ac in range(NAC):
            ups = sm_psum.tile([128, 128], F32, tag="smp")
            for dc in range(NDC):
                nc.tensor.matmul(ups[:, :b_loc],
                                 lhsT=wdecT_sb[:, dc, ac * 128:(ac + 1) * 128],
                                 rhs=dec_sb[:, dc, :],
                                 start=(dc == 0), stop=(dc == NDC - 1))
            nc.scalar.activation(out=u_sb[:, ac, :], in_=ups[:, :b_loc],
                                 func=mybir.ActivationFunctionType.Identity,
                                 bias=bsum_sb[:, ac:ac + 1], scale=1.0)

        for b in range(b_loc):
            # ---- load + cast (split so transposes start on partial data) ----
            nat = nat_pool.tile([128, NT, D_ENC], BF16, tag="nat")
            enc_b = enc_d.ap()[b].rearrange("(n p) e -> p n e", p=128)
            if b == 0:
                splits = [1, 1, 2, 4, 8, 8, 8]
            else:
                splits = [4] * 8
            pos = 0
            for w in splits:
                nc.gpsimd.dma_start(
                    out=nat[:, pos:pos + w, :],
                    in_=enc_b[:, pos:pos + w, :])
                pos += w

            # ---- transpose to encT ----
            encT = encT_pool.tile([128, NEC, t_len], BF16, tag="encT")
            for tt in range(NTT):
                for ec in range(NEC):
                    ps = tr_psum.tile([128, 512], BF16, tag="trps")
                    for q in range(4):
                        nc.tensor.transpose(
                            ps[:, q * 128:(q + 1) * 128],
                            nat[:, tt * 4 + q, ec * 128:(ec + 1) * 128],
                            idn_bf)
                    nc.vector.tensor_copy(
                        out=encT[:, ec, tt * 512:(tt + 1) * 512], in_=ps)

            # ---- enc_proj + tanh -> energy ----
            energy = en_pool.tile([128, NAC, t_len], BF16, tag="energy")
            for tt in range(NTT):
                for ac in range(NAC):
                    pps = ep_psum.tile([128, 512], F32, tag="epps")
                    for ec in range(NEC):
                        nc.tensor.matmul(
                            pps,
                            lhsT=wencT_sb[:, ec, ac * 128:(ac + 1) * 128],
                            rhs=encT[:, ec, tt * 512:(tt + 1) * 512],
                            start=(ec == 0), stop=(ec == NEC - 1))
                    nc.scalar.activation(
                        out=energy[:, ac, tt * 512:(tt + 1) * 512], in_=pps,
                        func=mybir.ActivationFunctionType.Tanh,
                        bias=u_sb[:, ac, b:b + 1], scale=1.0)

            # ---- scores^T [t%128, c] ----
            scps = sc_psum.tile([128, NT], F32, tag="scps")
            for c in range(NT):
                for ac in range(NAC):
                    nc.tensor.matmul(
                        scps[:, c:c + 1],
                        lhsT=energy[:, ac, c * 128:(c + 1) * 128],
                        rhs=v_bf[:, ac:ac + 1],
                        start=(ac == 0), stop=(ac == NAC - 1))

            # ---- softmax on [128, NT] ----
            p_sb = small_pool.tile([128, NT], F32, tag="p")
            rowsum = small_pool.tile([128, 1], F32, tag="rowsum")
            if apply_mask:
                mb = small_pool.tile([128, NT], F32, tag="mb")
                nc.sync.dma_start(out=mb, in_=mbias_d.ap()[b])
                nc.vector.tensor_add(p_sb, scps, mb)
                nc.scalar.activation(out=p_sb, in_=p_sb,
                                     func=mybir.ActivationFunctionType.Exp,
                                     accum_out=rowsum)
            else:
                nc.scalar.activation(out=p_sb, in_=scps,
                                     func=mybir.ActivationFunctionType.Exp,
                                     accum_out=rowsum)
            # total = sum over partitions of rowsum (PE), then 1/total
            tot_ps = sm_psum.tile([128, 128], F32, tag="smp")
            nc.tensor.matmul(tot_ps[:1, :1], lhsT=rowsum, rhs=ones_f32,
                             start=True, stop=True)
            sinv = small_pool.tile([1, 1], F32, tag="sinv")
            nc.vector.reciprocal(sinv, tot_ps[:1, :1])
            # broadcast 1/total to all partitions
            bc_ps = sm_psum.tile([128, 128], F32, tag="smp")
            nc.tensor.matmul(bc_ps[:, :1], lhsT=ones_row, rhs=sinv,
                             start=True, stop=True)
            sinv_bc = small_pool.tile([128, 1], F32, tag="sinvbc")
            nc.vector.tensor_copy(sinv_bc, bc_ps[:, :1])

            # ---- weights out: W = P * sinv, transpose, DMA ----
            w_sb = small_pool.tile([128, NT], F32, tag="w")
            nc.vector.tensor_scalar_mul(w_sb, p_sb, sinv_bc)
            wt_ps = sm_psum.tile([128, 128], F32, tag="smp")
            nc.tensor.transpose(wt_ps[:NT, :], w_sb, idn_f32)
            wT_sb = small_pool.tile([NT, 128], F32, tag="wT")
            nc.vector.tensor_copy(wT_sb, wt_ps[:NT, :])
            nc.sync.dma_start(
                out=wout_d.ap()[b].rearrange("(c p) -> c p", c=NT), in_=wT_sb)

            # ---- context: sum_t exp(s_t) * enc[t, :], scaled by sinv ----
            p_bf = small_pool.tile([128, NT], BF16, tag="pbf")
            nc.vector.tensor_copy(p_bf, p_sb)
            cxps = cx_psum.tile([1, D_ENC], F32, tag="cxps")
            for c in range(NT):
                nc.tensor.matmul(cxps, lhsT=p_bf[:, c:c + 1], rhs=nat[:, c, :],
                                 start=(c == 0), stop=(c == NT - 1))
            ctx_sb = small_pool.tile([1, D_ENC], F32, tag="ctx")
            nc.scalar.activation(out=ctx_sb, in_=cxps,
                                 func=mybir.ActivationFunctionType.Copy,
                                 scale=sinv, bias=0.0)
            nc.sync.dma_start(out=ctx_d.ap()[b:b + 1, :], in_=ctx_sb)

    nc.compile()
    _program_cache[key] = nc
    return nc


def _prep_shared(W_enc, b_enc, W_dec, b_dec, v):
    wencT = np.ascontiguousarray(W_enc.T).astype(ml_dtypes.bfloat16)
    wdecT = np.ascontiguousarray(W_dec.T).astype(np.float32)
    bsum = np.ascontiguousarray(
        (b_enc.astype(np.float32) + b_dec.astype(np.float32))
        .reshape(D_ATT // 128, 128).T)
    v2 = np.ascontiguousarray(
        v.astype(np.float32).reshape(D_ATT // 128, 128).T)
    return wencT, wdecT, bsum, v2


def kernel_impl(decoder_hidden, encoder_outputs, encoder_mask,
                W_enc, b_enc, W_dec, b_dec, v, trace=False, trace_kwargs=None):
    decoder_hidden = np.asarray(decoder_hidden, dtype=np.float32)
    encoder_outputs = np.asarray(encoder_outputs, dtype=np.float32)
    encoder_mask = np.asarray(encoder_mask)
    W_enc = np.asarray(W_enc, dtype=np.float32)
    b_enc = np.asarray(b_enc, dtype=np.float32)
    W_dec = np.asarray(W_dec, dtype=np.float32)
    b_dec = np.asarray(b_dec, dtype=np.float32)
    v = np.asarray(v, dtype=np.float32)

    apply_mask = not bool(encoder_mask.all())
    nc = build_program(B_LOC, T, apply_mask)
    wencT, wdecT, bsum, v2 = _prep_shared(W_enc, b_enc, W_dec, b_dec, v)

    in_maps = []
    for c in range(N_CORES):
        lo, hi = c * B_LOC, (c + 1) * B_LOC
        m = {
            "enc": np.ascontiguousarray(encoder_outputs[lo:hi]),
            "idnbf": np.eye(128, dtype=ml_dtypes.bfloat16),
            "idnf": np.eye(128, dtype=np.float32),
            "dect": np.ascontiguousarray(decoder_hidden[lo:hi].T),
            "wencT": wencT,
            "wdecT": wdecT,
            "bsum": bsum,
            "v": v2,
        }
        if apply_mask:
            mbias = np.where(encoder_mask[lo:hi], 0.0, -1e30).astype(np.float32)
            # scores^T layout: [b, t%128, t//128]
            m["mbias"] = np.ascontiguousarray(
                mbias.reshape(B_LOC, T // 128, 128).transpose(0, 2, 1))
        in_maps.append(m)

    res = bass_utils.run_bass_kernel_spmd(
        nc, in_maps, core_ids=list(range(N_CORES)), trace=trace,
        **(trace_kwargs or {}))
    ctx = np.concatenate([res.results[c]["ctx"] for c in range(N_CORES)], axis=0)
    wts = np.concatenate([res.results[c]["wout"] for c in range(N_CORES)], axis=0)
    return (ctx, wts), res


def kernel(decoder_hidden, encoder_outputs, encoder_mask,
           W_enc, b_enc, W_dec, b_dec, v):
    out, _ = kernel_impl(decoder_hidden, encoder_outputs, encoder_mask,
                         W_enc, b_enc, W_dec, b_dec, v)
    return out
